# revision 9
# baseline (speedup 1.0000x reference)
"""Trainium2 Bass kernel for nn_MemoryAttention (causal single-head attention
with SiLU-gated output projection), sequence-parallel across 8 NeuronCores.

Strategy (per core c):
  - q rows owned: 4 slots of 256 rows: tile T = c + 8*s (strided assignment
    balances causal work; every core runs an identical instruction stream).
  - Q/K path runs in fp8e4 with DoubleRow double-pumping: host pre-scales
    wq/wk by 32 (weights are ~1e-2 std, far below e4m3 normal range) and the
    exp() scale absorbs the extra 1024x. V path stays bf16.
  - Each core projects KT (fp8) / V (bf16) for its own 4 column-groups and
    AllGathers them per group (8 small collectives) so the first groups
    arrive while later groups still project.
  - Each slot visits its two *diagonal* kv blocks first — those are locally
    produced (no collective wait) — then gathered blocks 0..16s+13.
    Gathered visits are all-visible or all-hidden per core; that mask is
    folded into the exp() bias (0 or -30000) streamed as a per-partition
    scalar, so no mask tensors are loaded per visit. The two diagonal visits
    use two resident triangular masks.
  - Per visit: LT[kv, q] = K @ QT in PSUM (4 fp8 DoubleRow matmuls),
    PT = exp(LT * 2^-15 + bias), then PT q-chunks become the stationary
    operand for both H[q, d] += P @ V (N=512) and rowsums += P @ 1 (N=1).
  - Slot epilogue: H / sums (per-partition scalar), SiLU, PE-transpose of G,
    output projection with G^T chunks stationary -> O[q, d] written directly.
"""

import numpy as np
import ml_dtypes

import concourse.bass as bass
import concourse.tile as tile
from concourse import bacc, mybir
from concourse.bass_utils import run_bass_kernel_spmd
from concourse.masks import make_identity

P = 128
D = 1024
SEQ = 8192
NCORES = 8
NSLOTS = 4
QT_COLS = NSLOTS * 256
NBIN = 14  # gathered visits per slot that need a (binary) mask on some core
KSCALE = 32.0

F32 = mybir.dt.float32
BF16 = mybir.dt.bfloat16
F8 = mybir.dt.float8e4
AF = mybir.ActivationFunctionType
DR = mybir.MatmulPerfMode.DoubleRow
EXP_SCALE = 1.0 / (KSCALE * KSCALE * 32.0)  # logits carry 32q * 32k


def build_kernel():
    nc = bacc.Bacc(None, target_bir_lowering=False, num_devices=NCORES)

    xq8_ext = nc.declare_dram_parameter("xq8", [D, QT_COLS], F8, isOutput=False)
    xqb_ext = nc.declare_dram_parameter("xqb", [D, QT_COLS], BF16, isOutput=False)
    wq_ext = nc.declare_dram_parameter("wq8", [D, D], F8, isOutput=False)
    wk_ext = nc.declare_dram_parameter("wk8", [D, D], F8, isOutput=False)
    wv1_ext = nc.declare_dram_parameter("wv1", [D, D], BF16, isOutput=False)
    wv2_ext = nc.declare_dram_parameter("wv2", [D, D], BF16, isOutput=False)
    dmask_ext = nc.declare_dram_parameter("dmask", [P, 2, 256], BF16, isOutput=False)
    bias_ext = nc.declare_dram_parameter("bias", [P, NSLOTS * NBIN], F32, isOutput=False)
    o_ext = nc.declare_dram_parameter("o", [NSLOTS, 2, P, D], F32, isOutput=True)

    # own kv payload per 256-col group g: [parity][p][m][c]
    kt_loc = nc.dram_tensor("kt_loc", [4, 2, P, 8, P], F8)
    v_loc = nc.dram_tensor("v_loc", [4, 2, P, 8, P], BF16)
    # groups 0/1 gathered individually (arrive early), 2+3 merged
    kt_g0 = nc.dram_tensor("kt_g0", [NCORES, 2, P, 8, P], F8, addr_space="Shared")
    v_g0 = nc.dram_tensor("v_g0", [NCORES, 2, P, 8, P], BF16, addr_space="Shared")
    kt_g1 = nc.dram_tensor("kt_g1", [NCORES, 2, P, 8, P], F8, addr_space="Shared")
    v_g1 = nc.dram_tensor("v_g1", [NCORES, 2, P, 8, P], BF16, addr_space="Shared")
    kt_g23 = nc.dram_tensor("kt_g23", [NCORES, 2, 2, P, 8, P], F8, addr_space="Shared")
    v_g23 = nc.dram_tensor("v_g23", [NCORES, 2, 2, P, 8, P], BF16, addr_space="Shared")

    with tile.TileContext(nc) as tc:
        singles_ctx = tc.tile_pool(name="singles", bufs=1)
        singles = singles_ctx.__enter__()

        with (
            tc.tile_pool(name="projw", bufs=1) as projw,
            tc.tile_pool(name="projout", bufs=4) as projout,
            tc.tile_pool(name="ppsum", bufs=4, space="PSUM") as ppsum,
            tc.tile_pool(name="kpsum", bufs=4, space="PSUM") as kpsum,
        ):
            # chunked loads ordered so the first kt-proj matmuls start early;
            # spread across both HWDGE queues (sync + scalar) to halve the
            # startup DMA serialization
            wk_sb = projw.tile([P, 8, D], F8, tag="wk", name="wk")
            wk_v = wk_ext[:].rearrange("(sub p) s -> p sub s", p=P)
            xq8_sb = singles.tile([P, 8, QT_COLS], F8)
            xq8_v = xq8_ext[:].rearrange("(sub p) s -> p sub s", p=P)
            nc.scalar.dma_start(out=xq8_sb[:, :, :256], in_=xq8_v[:, :, :256])
            for m in range(8):
                nc.sync.dma_start(
                    out=wk_sb[:, :, m * P : (m + 1) * P],
                    in_=wk_v[:, :, m * P : (m + 1) * P],
                )
            xqb_sb = projw.tile([P, 8, QT_COLS], BF16, tag="xqb", name="xqb")
            xqb_v = xqb_ext[:].rearrange("(sub p) s -> p sub s", p=P)
            nc.scalar.dma_start(out=xqb_sb[:, :, :256], in_=xqb_v[:, :, :256])
            wv1_sb = projw.tile([P, 8, D], BF16, tag="wv1", name="wv1")
            wv1_v = wv1_ext[:].rearrange("(sub p) s -> p sub s", p=P)
            for h2 in range(2):
                nc.scalar.dma_start(
                    out=wv1_sb[:, :, h2 * 512 : (h2 + 1) * 512],
                    in_=wv1_v[:, :, h2 * 512 : (h2 + 1) * 512],
                )
            nc.sync.dma_start(out=xq8_sb[:, :, 256:], in_=xq8_v[:, :, 256:])
            nc.scalar.dma_start(out=xqb_sb[:, :, 256:], in_=xqb_v[:, :, 256:])
            wq_sb = singles.tile([P, 8, D], F8)
            nc.sync.dma_start(
                out=wq_sb, in_=wq_ext[:].rearrange("(sub p) s -> p sub s", p=P)
            )

            ones_sb = singles.tile([P, 1], BF16)
            nc.vector.memset(ones_sb, 1.0)
            zcol_sb = singles.tile([1, P], BF16)
            nc.vector.memset(zcol_sb, 0.0)
            zrow_sb = singles.tile([1, 2], BF16)
            nc.vector.memset(zrow_sb, 0.0)
            ident_sb = singles.tile([P, P], BF16)
            make_identity(nc, ident_sb)

            def gather(ins_ap, outs_ap):
                nc.gpsimd.collective_compute(
                    "AllGather",
                    mybir.AluOpType.bypass,
                    replica_groups=[list(range(NCORES))],
                    ins=[ins_ap],
                    outs=[outs_ap],
                )

            def kt_group(g):
                # KT for own group g: out [d_out(m,p), kv 256] in fp8
                cols = slice(g * 256, (g + 1) * 256)
                for m in range(8):
                    acc = kpsum.tile([P, 256], F32, tag="projk", name=f"kt{g}_{m}")
                    for sp in range(4):
                        nc.tensor.matmul(
                            acc,
                            lhsT=wk_sb[:, 2 * sp : 2 * sp + 2, m * P : (m + 1) * P],
                            rhs=xq8_sb[:, 2 * sp : 2 * sp + 2, cols],
                            start=(sp == 0),
                            stop=(sp == 3),
                            perf_mode=DR,
                        )
                    kt_out = projout.tile([P, 256], F8, tag="kt_out", name="kto")
                    nc.vector.tensor_copy(out=kt_out, in_=acc)
                    for par in range(2):
                        nc.sync.dma_start(
                            out=kt_loc[g, par][:, m, :],
                            in_=kt_out[:, par * P : (par + 1) * P],
                        )

            def v_group(g):
                # V for own group g (bf16)
                for par in range(2):
                    v_out = projout.tile([P, 1024], BF16, tag="v_out", name="vo")
                    accs = [
                        ppsum.tile([P, 512], F32, tag="proj", name=f"vp{h2}")
                        for h2 in range(2)
                    ]
                    xcol = slice(g * 256 + par * P, g * 256 + (par + 1) * P)
                    for sub in range(8):
                        for h2 in range(2):
                            nc.tensor.matmul(
                                accs[h2],
                                lhsT=xqb_sb[:, sub, xcol],
                                rhs=wv1_sb[:, sub, h2 * 512 : (h2 + 1) * 512],
                                start=(sub == 0),
                                stop=(sub == 7),
                            )
                    for h2 in range(2):
                        nc.vector.tensor_copy(
                            out=v_out[:, h2 * 512 : (h2 + 1) * 512], in_=accs[h2]
                        )
                    nc.sync.dma_start(
                        out=v_loc[g, par].rearrange("p m c -> p (m c)"), in_=v_out
                    )

            qt_sb = singles.tile([P, 8, QT_COLS], F8)

            def q_proj(n0, n1):
                # QT (fp8 DoubleRow) for 512-col chunks n0..n1
                for m in range(8):
                    accs = [
                        ppsum.tile([P, 512], F32, tag="proj", name=f"qp{n}")
                        for n in range(n0, n1)
                    ]
                    for sp in range(4):
                        for i, n in enumerate(range(n0, n1)):
                            nc.tensor.matmul(
                                accs[i],
                                lhsT=wq_sb[:, 2 * sp : 2 * sp + 2, m * P : (m + 1) * P],
                                rhs=xq8_sb[
                                    :, 2 * sp : 2 * sp + 2, n * 512 : (n + 1) * 512
                                ],
                                start=(sp == 0),
                                stop=(sp == 3),
                                perf_mode=DR,
                            )
                    for i, n in enumerate(range(n0, n1)):
                        nc.vector.tensor_copy(
                            out=qt_sb[:, m, n * 512 : (n + 1) * 512], in_=accs[i]
                        )

            kt_group(0)
            gather(kt_loc[0], kt_g0[:])
            v_group(0)
            gather(v_loc[0], v_g0[:])
            kt_group(1)
            gather(kt_loc[1], kt_g1[:])
            v_group(1)
            gather(v_loc[1], v_g1[:])
            kt_group(2)
            v_group(2)
            kt_group(3)
            v_group(3)
            gather(kt_loc[2:4], kt_g23[:])
            gather(v_loc[2:4], v_g23[:])

            # QT for slot 0 (cols 0:512 cover slots 0 and 1); the rest is
            # emitted inside the attention section as collective-wait filler.
            q_proj(0, 1)

        # ---- attention ----------------------------------------------------
        with (
            tc.tile_pool(name="asingles", bufs=1) as asingles,
            tc.tile_pool(name="vpool", bufs=10) as vpool,
            tc.tile_pool(name="epool", bufs=2) as epool,
            tc.tile_pool(name="gpool", bufs=2) as gpool,
            tc.tile_pool(name="ltpsum", bufs=2, space="PSUM") as ltpsum,
            tc.tile_pool(name="hpsum", bufs=1, space="PSUM") as hpsum,
            tc.tile_pool(name="spsum", bufs=1, space="PSUM") as spsum,
            tc.tile_pool(name="tppsum", bufs=1, space="PSUM") as tppsum,
        ):
            wv2_sb = asingles.tile([P, 8, D], BF16, tag="wv2", name="wv2")
            nc.sync.dma_start(
                out=wv2_sb, in_=wv2_ext[:].rearrange("(sub p) s -> p sub s", p=P)
            )
            dm_sb = asingles.tile([P, 2, 256], BF16, tag="dm", name="dm")
            nc.sync.dma_start(out=dm_sb, in_=dmask_ext[:])
            bias_sb = asingles.tile([P, NSLOTS * NBIN], F32, tag="bias", name="bias")
            nc.sync.dma_start(out=bias_sb, in_=bias_ext[:])

            def visit_srcs(s, kind, idx):
                if kind == "diag":
                    return kt_loc[s, idx], v_loc[s, idx]
                g, src, par = idx // 16, (idx % 16) // 2, idx % 2
                if g == 0:
                    return kt_g0[src, par], v_g0[src, par]
                if g == 1:
                    return kt_g1[src, par], v_g1[src, par]
                return kt_g23[src, g - 2, par], v_g23[src, g - 2, par]

            def q_proj_filler():
                # QT cols 512:1024 (slots 2/3), emitted as stall filler while
                # slot 0 waits for the first gathers; accumulates in the lt pool
                for m in range(8):
                    for n4 in range(2, 4):
                        acc = ltpsum.tile([P, 256], F32, tag="lt", name=f"qf{m}_{n4}")
                        for sp in range(4):
                            nc.tensor.matmul(
                                acc,
                                lhsT=wq_sb[:, 2 * sp : 2 * sp + 2, m * P : (m + 1) * P],
                                rhs=xq8_sb[
                                    :, 2 * sp : 2 * sp + 2, n4 * 256 : (n4 + 1) * 256
                                ],
                                start=(sp == 0),
                                stop=(sp == 3),
                                perf_mode=DR,
                            )
                        nc.vector.tensor_copy(
                            out=qt_sb[:, m, n4 * 256 : (n4 + 1) * 256], in_=acc
                        )

            def load_visit(s, kind, idx):
                kt_src, v_src = visit_srcs(s, kind, idx)
                kt_t = vpool.tile([P, 8, P], F8, tag="kt", name="kt_t")
                nc.sync.dma_start(out=kt_t, in_=kt_src)
                v_t = vpool.tile([P, 1024], BF16, tag="v", name="v_t")
                nc.scalar.dma_start(out=v_t, in_=v_src.rearrange("p m c -> p (m c)"))
                return kt_t, v_t

            def logits(s, kt_t):
                lt = ltpsum.tile([P, 256], F32, tag="lt", name="lt")
                for sp in range(4):
                    nc.tensor.matmul(
                        lt,
                        lhsT=kt_t[:, 2 * sp : 2 * sp + 2, :],
                        rhs=qt_sb[:, 2 * sp : 2 * sp + 2, s * 256 : (s + 1) * 256],
                        start=(sp == 0),
                        stop=(sp == 3),
                        perf_mode=DR,
                    )
                return lt

            def pv(s, j, kind, idx, lt, v_t, h, sums, jmax):
                pt = vpool.tile([P, 256], BF16, tag="pt", name="pt")
                if kind == "gath" and idx >= 16 * s:
                    bi = s * NBIN + (idx - 16 * s)
                    bias = bias_sb[:, bi : bi + 1]
                else:
                    bias = 0.0
                nc.scalar.activation(
                    out=pt, in_=lt, func=AF.Exp, scale=EXP_SCALE, bias=bias
                )
                if kind == "diag":
                    nc.vector.tensor_mul(out=pt, in0=pt, in1=dm_sb[:, idx, :])
                for qc in range(2):
                    lhsT = pt[:, qc * P : (qc + 1) * P]
                    for dh in range(2):
                        nc.tensor.matmul(
                            h[qc][:, dh, :],
                            lhsT=lhsT,
                            rhs=v_t[:, dh * 512 : (dh + 1) * 512],
                            start=(j == 0),
                            stop=(j == jmax),
                        )
                    nc.tensor.matmul(
                        sums[:, qc : qc + 1],
                        lhsT=lhsT,
                        rhs=ones_sb,
                        start=False,
                        stop=(j == jmax),
                        skip_group_check=True,
                    )

            for s in range(NSLOTS):
                visits = [("diag", 0), ("diag", 1)] + [
                    ("gath", b) for b in range(16 * s + NBIN)
                ]
                jmax = len(visits) - 1
                h = [
                    hpsum.tile([P, 2, 512], F32, tag=f"hq{qc}", name=f"h{qc}_{s}")
                    for qc in range(2)
                ]
                sums = spsum.tile([P, 2], F32, tag="sums", name="sums")
                nc.tensor.matmul(
                    sums,
                    lhsT=zcol_sb,
                    rhs=zrow_sb,
                    start=True,
                    stop=False,
                    skip_group_check=True,
                )
                # software pipeline: logits of visit j+1 are emitted before pv of j
                kt_t, v_t = load_visit(s, *visits[0])
                lt_prev = logits(s, kt_t)
                v_prev = v_t
                for j in range(1, len(visits)):
                    kt_t, v_t = load_visit(s, *visits[j])
                    lt = logits(s, kt_t)
                    pv(s, j - 1, *visits[j - 1], lt_prev, v_prev, h, sums, jmax)
                    if s == 0 and j == 1:
                        q_proj_filler()
                    lt_prev, v_prev = lt, v_t
                pv(s, jmax, *visits[jmax], lt_prev, v_prev, h, sums, jmax)

                # ---- epilogue (chained per qc so qc0's output projection
                # overlaps qc1's vector/scalar work) -----------------------
                gt_sb = epool.tile([P, 8, 256], BF16, tag="gt", name="gt")
                for qc in range(2):
                    recip = epool.tile([P, 1], F32, tag="recip", name="recip")
                    nc.vector.reciprocal(out=recip, in_=sums[:, qc : qc + 1])
                    g32 = epool.tile([P, 2, 512], F32, tag="g32", name="g32")
                    nc.vector.tensor_scalar_mul(out=g32, in0=h[qc], scalar1=recip)
                    gb = gpool.tile([P, 1024], BF16, tag=f"g{qc}", name=f"g{qc}")
                    nc.scalar.activation(
                        out=gb, in_=g32.rearrange("p a b -> p (a b)"), func=AF.Silu
                    )
                    # transpose G -> gt [d-part, m, qc*P:...]
                    for m in range(8):
                        tp = tppsum.tile([P, 256], BF16, tag="tp", name="tp")
                        nc.tensor.transpose(
                            tp[:, :P],
                            gb[:, m * P : (m + 1) * P],
                            ident_sb,
                        )
                        nc.vector.tensor_copy(
                            out=gt_sb[:, m, qc * P : (qc + 1) * P], in_=tp[:, :P]
                        )
                    # output projection: O[q, d] via lhsT = gt chunks
                    op = hpsum.tile(
                        [P, 2, 512], F32, tag=f"hq{qc}", name=f"o{qc}_{s}"
                    )
                    for m in range(8):
                        for dh in range(2):
                            nc.tensor.matmul(
                                op[:, dh, :],
                                lhsT=gt_sb[:, m, qc * P : (qc + 1) * P],
                                rhs=wv2_sb[:, m, dh * 512 : (dh + 1) * 512],
                                start=(m == 0),
                                stop=(m == 7),
                            )
                    oo = epool.tile([P, 2, 512], F32, tag="oo", name="oo")
                    nc.vector.tensor_copy(out=oo, in_=op)
                    nc.sync.dma_start(
                        out=o_ext[s, qc], in_=oo.rearrange("p a b -> p (a b)")
                    )

        singles_ctx.__exit__(None, None, None)

    nc.finalize()
    return nc


_NC_CACHE = {}


def get_nc():
    if "nc" not in _NC_CACHE:
        _NC_CACHE["nc"] = build_kernel()
    return _NC_CACHE["nc"]


def build_dmask():
    p = np.arange(P)[:, None]
    u = np.arange(256)[None, :]
    m0 = (p <= u).astype(np.float32)
    m1 = (p + P <= u).astype(np.float32)
    return np.stack([m0, m1], axis=1).astype(ml_dtypes.bfloat16)  # [P, 2, 256]


def build_bias(c):
    """exp-bias per gathered-masked visit: slot s, i = block - 16s in [0, 14):
    visible on core c iff i < 2c."""
    row = np.empty(NSLOTS * NBIN, np.float32)
    for s in range(NSLOTS):
        for i in range(NBIN):
            row[s * NBIN + i] = 0.0 if i < 2 * c else -30000.0
    return np.broadcast_to(row, (P, NSLOTS * NBIN)).copy()


def build_in_maps(x, wq, wk, wv1, wv2):
    bf = ml_dtypes.bfloat16
    f8 = ml_dtypes.float8_e4m3fn
    xT = np.ascontiguousarray(np.asarray(x, np.float32).T)
    dmask = build_dmask()

    def to8(a):
        return np.clip(np.asarray(a, np.float32), -240, 240).astype(f8)

    w = {
        "wq8": to8(np.asarray(wq, np.float32) * KSCALE),
        "wk8": to8(np.asarray(wk, np.float32) * KSCALE),
        "wv1": np.asarray(wv1, np.float32).astype(bf),
        "wv2": np.asarray(wv2, np.float32).astype(bf),
        "dmask": dmask,
    }
    in_maps = []
    for c in range(NCORES):
        xq_c = np.concatenate(
            [xT[:, 256 * (c + 8 * s) : 256 * (c + 8 * s) + 256] for s in range(NSLOTS)],
            axis=1,
        )
        in_maps.append(
            {
                "xq8": to8(xq_c),
                "xqb": np.ascontiguousarray(xq_c).astype(bf),
                "bias": build_bias(c),
                **w,
            }
        )
    return in_maps


def assemble_out(results):
    out = np.empty((SEQ, D), np.float32)
    for c in range(NCORES):
        o = results[c]["o"]  # [4, 2, 128, 1024]
        for s in range(NSLOTS):
            r0 = 256 * (c + 8 * s)
            out[r0 : r0 + P, :] = o[s, 0]
            out[r0 + P : r0 + 256, :] = o[s, 1]
    return out


def kernel(x, wq, wk, wv1, wv2):
    in_maps = build_in_maps(x, wq, wk, wv1, wv2)
    nc = get_nc()
    res = run_bass_kernel_spmd(nc, in_maps, list(range(NCORES)))
    return assemble_out(res.results)


# revision 15
# speedup vs baseline: 1.0396x; 1.0396x over previous
"""Trainium2 Bass kernel for nn_MemoryAttention (causal single-head attention
with SiLU-gated output projection), sequence-parallel across 8 NeuronCores.

Strategy (per core c):
  - q rows owned: 4 slots of 256 rows: tile T = c + 8*s (strided assignment
    balances causal work; every core runs an identical instruction stream).
  - Q/K path runs in fp8e4 with DoubleRow double-pumping: host pre-scales
    wq/wk by 32 (weights are ~1e-2 std, far below e4m3 normal range) and the
    exp() scale absorbs the extra 1024x. V path stays bf16.
  - Each core projects KT (fp8) / V (bf16) for its own 4 column-groups and
    AllGathers them per group (8 small collectives) so the first groups
    arrive while later groups still project.
  - Each slot visits its two *diagonal* kv blocks first — those are locally
    produced (no collective wait) — then gathered blocks 0..16s+13.
    Gathered visits are all-visible or all-hidden per core; that mask is
    folded into the exp() bias (0 or -30000) streamed as a per-partition
    scalar, so no mask tensors are loaded per visit. The two diagonal visits
    use two resident triangular masks.
  - Per visit: LT[kv, q] = K @ QT in PSUM (4 fp8 DoubleRow matmuls),
    PT = exp(LT * 2^-15 + bias), then PT q-chunks become the stationary
    operand for both H[q, d] += P @ V (N=512) and rowsums += P @ 1 (N=1).
  - Slot epilogue: H / sums (per-partition scalar), SiLU, PE-transpose of G,
    output projection with G^T chunks stationary -> O[q, d] written directly.
"""

import numpy as np
import ml_dtypes

import concourse.bass as bass
import concourse.tile as tile
from concourse import bacc, mybir
from concourse.bass_utils import run_bass_kernel_spmd
from concourse.masks import make_identity

P = 128
D = 1024
SEQ = 8192
NCORES = 8
NSLOTS = 4
QT_COLS = NSLOTS * 256
NBIN = 14  # gathered visits per slot that need a (binary) mask on some core
KSCALE = 32.0

F32 = mybir.dt.float32
BF16 = mybir.dt.bfloat16
F8 = mybir.dt.float8e4
AF = mybir.ActivationFunctionType
DR = mybir.MatmulPerfMode.DoubleRow
EXP_SCALE = 1.0 / (KSCALE * KSCALE * 32.0)  # logits carry 32q * 32k


def build_kernel():
    nc = bacc.Bacc(None, target_bir_lowering=False, num_devices=NCORES)

    xq8_ext = nc.declare_dram_parameter("xq8", [D, QT_COLS], F8, isOutput=False)
    xqb_ext = nc.declare_dram_parameter("xqb", [D, QT_COLS], BF16, isOutput=False)
    wq_ext = nc.declare_dram_parameter("wq8", [D, D], F8, isOutput=False)
    wk_ext = nc.declare_dram_parameter("wk8", [D, D], F8, isOutput=False)
    wv1_ext = nc.declare_dram_parameter("wv1", [D, D], BF16, isOutput=False)
    wv2_ext = nc.declare_dram_parameter("wv2", [D, D], BF16, isOutput=False)
    dmask_ext = nc.declare_dram_parameter("dmask", [P, 2, 256], BF16, isOutput=False)
    bias_ext = nc.declare_dram_parameter("bias", [P, NSLOTS * NBIN], F32, isOutput=False)
    o_ext = nc.declare_dram_parameter("o", [NSLOTS, 2, P, D], F32, isOutput=True)

    # own kv payload per 256-col group g: [parity][p][m][c]. Separate tensors
    # per group: the tile framework tracks DRAM deps at tensor granularity,
    # so a shared tensor would delay the first gather until ALL groups project.
    kt_loc = [nc.dram_tensor(f"kt_loc{g}", [2, P, 8, P], F8) for g in range(4)]
    v_loc = [nc.dram_tensor(f"v_loc{g}", [2, P, 8, P], BF16) for g in range(4)]
    kt_loc23 = nc.dram_tensor("kt_loc23", [2, 2, P, 8, P], F8)
    kt_gth = [
        nc.dram_tensor(f"kt_g{g}", [NCORES, 2, P, 8, P], F8, addr_space="Shared")
        for g in range(2)
    ]
    v_gth = [
        nc.dram_tensor(f"v_g{g}", [NCORES, 2, P, 8, P], BF16, addr_space="Shared")
        for g in range(4)
    ]
    kt_g23 = nc.dram_tensor("kt_g23", [NCORES, 2, 2, P, 8, P], F8, addr_space="Shared")

    with tile.TileContext(nc) as tc:
        singles_ctx = tc.tile_pool(name="singles", bufs=1)
        singles = singles_ctx.__enter__()

        with (
            tc.tile_pool(name="projw", bufs=1) as projw,
            tc.tile_pool(name="projout", bufs=4) as projout,
            tc.tile_pool(name="ppsum", bufs=4, space="PSUM") as ppsum,
            tc.tile_pool(name="kpsum", bufs=4, space="PSUM") as kpsum,
        ):
            # chunked loads ordered so the first kt-proj matmuls start early;
            # spread across both HWDGE queues (sync + scalar) to halve the
            # startup DMA serialization
            wk_sb = projw.tile([P, 8, D], F8, tag="wk", name="wk")
            wk_v = wk_ext[:].rearrange("(sub p) s -> p sub s", p=P)
            xq8_sb = singles.tile([P, 8, QT_COLS], F8)
            xq8_v = xq8_ext[:].rearrange("(sub p) s -> p sub s", p=P)
            nc.sync.dma_start(out=xq8_sb[:, :, :256], in_=xq8_v[:, :, :256])
            for m in range(8):
                nc.sync.dma_start(
                    out=wk_sb[:, :, m * P : (m + 1) * P],
                    in_=wk_v[:, :, m * P : (m + 1) * P],
                )
            wv1_sb = projw.tile([P, 8, D], BF16, tag="wv1", name="wv1")
            wv1_v = wv1_ext[:].rearrange("(sub p) s -> p sub s", p=P)
            nc.scalar.dma_start(
                out=wv1_sb[:, :, :512], in_=wv1_v[:, :, :512]
            )
            xqb_sb = projw.tile([P, 8, QT_COLS], BF16, tag="xqb", name="xqb")
            xqb_v = xqb_ext[:].rearrange("(sub p) s -> p sub s", p=P)
            nc.scalar.dma_start(out=xqb_sb[:, :, :256], in_=xqb_v[:, :, :256])
            nc.scalar.dma_start(out=wv1_sb[:, :, 512:], in_=wv1_v[:, :, 512:])
            nc.sync.dma_start(out=xq8_sb[:, :, 256:], in_=xq8_v[:, :, 256:])
            nc.scalar.dma_start(out=xqb_sb[:, :, 256:], in_=xqb_v[:, :, 256:])
            wq_sb = singles.tile([P, 8, D], F8)
            nc.sync.dma_start(
                out=wq_sb, in_=wq_ext[:].rearrange("(sub p) s -> p sub s", p=P)
            )

            ones_sb = singles.tile([P, 1], BF16)
            nc.vector.memset(ones_sb, 1.0)
            zcol_sb = singles.tile([1, P], BF16)
            nc.vector.memset(zcol_sb, 0.0)
            zrow_sb = singles.tile([1, 2], BF16)
            nc.vector.memset(zrow_sb, 0.0)
            ident_sb = singles.tile([P, P], BF16)
            make_identity(nc, ident_sb)

            def gather(ins_ap, outs_ap):
                nc.gpsimd.collective_compute(
                    "AllGather",
                    mybir.AluOpType.bypass,
                    replica_groups=[list(range(NCORES))],
                    ins=[ins_ap],
                    outs=[outs_ap],
                )

            def kt_group(g):
                # KT for own group g: out [d_out(m,p), kv 256] in fp8
                cols = slice(g * 256, (g + 1) * 256)
                dst = kt_loc[g] if g < 2 else kt_loc23[g - 2]
                for m in range(8):
                    acc = kpsum.tile([P, 256], F32, tag="projk", name=f"kt{g}_{m}")
                    for sp in range(4):
                        nc.tensor.matmul(
                            acc,
                            lhsT=wk_sb[:, 2 * sp : 2 * sp + 2, m * P : (m + 1) * P],
                            rhs=xq8_sb[:, 2 * sp : 2 * sp + 2, cols],
                            start=(sp == 0),
                            stop=(sp == 3),
                            perf_mode=DR,
                        )
                    kt_out = projout.tile([P, 256], F8, tag="kt_out", name="kto")
                    nc.vector.tensor_copy(out=kt_out, in_=acc)
                    for par in range(2):
                        nc.sync.dma_start(
                            out=dst[par][:, m, :],
                            in_=kt_out[:, par * P : (par + 1) * P],
                        )

            def v_group(g):
                # V for own group g (bf16)
                for par in range(2):
                    v_out = projout.tile([P, 1024], BF16, tag="v_out", name="vo")
                    accs = [
                        ppsum.tile([P, 512], F32, tag="proj", name=f"vp{h2}")
                        for h2 in range(2)
                    ]
                    xcol = slice(g * 256 + par * P, g * 256 + (par + 1) * P)
                    for sub in range(8):
                        for h2 in range(2):
                            nc.tensor.matmul(
                                accs[h2],
                                lhsT=xqb_sb[:, sub, xcol],
                                rhs=wv1_sb[:, sub, h2 * 512 : (h2 + 1) * 512],
                                start=(sub == 0),
                                stop=(sub == 7),
                            )
                    for h2 in range(2):
                        nc.vector.tensor_copy(
                            out=v_out[:, h2 * 512 : (h2 + 1) * 512], in_=accs[h2]
                        )
                    nc.sync.dma_start(
                        out=v_loc[g][par].rearrange("p m c -> p (m c)"), in_=v_out
                    )

            qt_sb = singles.tile([P, 8, QT_COLS], F8)

            def q_proj(n0, n1):
                # QT (fp8 DoubleRow) for 512-col chunks n0..n1
                for m in range(8):
                    accs = [
                        ppsum.tile([P, 512], F32, tag="proj", name=f"qp{n}")
                        for n in range(n0, n1)
                    ]
                    for sp in range(4):
                        for i, n in enumerate(range(n0, n1)):
                            nc.tensor.matmul(
                                accs[i],
                                lhsT=wq_sb[:, 2 * sp : 2 * sp + 2, m * P : (m + 1) * P],
                                rhs=xq8_sb[
                                    :, 2 * sp : 2 * sp + 2, n * 512 : (n + 1) * 512
                                ],
                                start=(sp == 0),
                                stop=(sp == 3),
                                perf_mode=DR,
                            )
                    for i, n in enumerate(range(n0, n1)):
                        nc.vector.tensor_copy(
                            out=qt_sb[:, m, n * 512 : (n + 1) * 512], in_=accs[i]
                        )

            kt_group(0)
            gather(kt_loc[0][:], kt_gth[0][:])
            v_group(0)
            gather(v_loc[0][:], v_gth[0][:])
            kt_group(1)
            gather(kt_loc[1][:], kt_gth[1][:])
            v_group(1)
            gather(v_loc[1][:], v_gth[1][:])
            kt_group(2)
            kt_group(3)
            gather(kt_loc23[:], kt_g23[:])
            v_group(2)
            gather(v_loc[2][:], v_gth[2][:])
            v_group(3)
            gather(v_loc[3][:], v_gth[3][:])

            # QT for slot 0 (cols 0:512 cover slots 0 and 1); the rest is
            # emitted inside the attention section as collective-wait filler.
            q_proj(0, 1)

        # ---- attention ----------------------------------------------------
        with (
            tc.tile_pool(name="asingles", bufs=1) as asingles,
            tc.tile_pool(name="vpool", bufs=10) as vpool,
            tc.tile_pool(name="epool", bufs=2) as epool,
            tc.tile_pool(name="gpool", bufs=2) as gpool,
            tc.tile_pool(name="ltpsum", bufs=2, space="PSUM") as ltpsum,
            tc.tile_pool(name="hpsum", bufs=1, space="PSUM") as hpsum,
            tc.tile_pool(name="spsum", bufs=1, space="PSUM") as spsum,
            tc.tile_pool(name="tppsum", bufs=1, space="PSUM") as tppsum,
        ):
            wv2_sb = asingles.tile([P, 8, D], BF16, tag="wv2", name="wv2")
            nc.sync.dma_start(
                out=wv2_sb, in_=wv2_ext[:].rearrange("(sub p) s -> p sub s", p=P)
            )
            dm_sb = asingles.tile([P, 2, 256], BF16, tag="dm", name="dm")
            nc.sync.dma_start(out=dm_sb, in_=dmask_ext[:])
            bias_sb = asingles.tile([P, NSLOTS * NBIN], F32, tag="bias", name="bias")
            nc.sync.dma_start(out=bias_sb, in_=bias_ext[:])

            def visit_srcs(s, kind, idx):
                if kind == "diag":
                    kt_src = kt_loc[s][idx] if s < 2 else kt_loc23[s - 2, idx]
                    return kt_src, v_loc[s][idx]
                g, src, par = idx // 16, (idx % 16) // 2, idx % 2
                kt_src = kt_gth[g][src, par] if g < 2 else kt_g23[src, g - 2, par]
                return kt_src, v_gth[g][src, par]

            def q_proj_filler():
                # QT cols 512:1024 (slots 2/3), emitted as stall filler while
                # slot 0 waits for the first gathers; accumulates in the lt pool
                for m in range(8):
                    for n4 in range(2, 4):
                        acc = ltpsum.tile([P, 256], F32, tag="lt", name=f"qf{m}_{n4}")
                        for sp in range(4):
                            nc.tensor.matmul(
                                acc,
                                lhsT=wq_sb[:, 2 * sp : 2 * sp + 2, m * P : (m + 1) * P],
                                rhs=xq8_sb[
                                    :, 2 * sp : 2 * sp + 2, n4 * 256 : (n4 + 1) * 256
                                ],
                                start=(sp == 0),
                                stop=(sp == 3),
                                perf_mode=DR,
                            )
                        nc.vector.tensor_copy(
                            out=qt_sb[:, m, n4 * 256 : (n4 + 1) * 256], in_=acc
                        )

            def load_visit(s, kind, idx):
                kt_src, v_src = visit_srcs(s, kind, idx)
                kt_t = vpool.tile([P, 8, P], F8, tag="kt", name="kt_t")
                nc.sync.dma_start(out=kt_t, in_=kt_src)
                v_t = vpool.tile([P, 1024], BF16, tag="v", name="v_t")
                nc.scalar.dma_start(out=v_t, in_=v_src.rearrange("p m c -> p (m c)"))
                return kt_t, v_t

            def logits(s, kt_t):
                lt = ltpsum.tile([P, 256], F32, tag="lt", name="lt")
                for sp in range(4):
                    nc.tensor.matmul(
                        lt,
                        lhsT=kt_t[:, 2 * sp : 2 * sp + 2, :],
                        rhs=qt_sb[:, 2 * sp : 2 * sp + 2, s * 256 : (s + 1) * 256],
                        start=(sp == 0),
                        stop=(sp == 3),
                        perf_mode=DR,
                    )
                return lt

            def pv(s, j, kind, idx, lt, v_t, h, sums, jmax):
                pt = vpool.tile([P, 256], BF16, tag="pt", name="pt")
                if kind == "gath" and idx >= 16 * s:
                    bi = s * NBIN + (idx - 16 * s)
                    bias = bias_sb[:, bi : bi + 1]
                else:
                    bias = 0.0
                nc.scalar.activation(
                    out=pt, in_=lt, func=AF.Exp, scale=EXP_SCALE, bias=bias
                )
                if kind == "diag":
                    nc.vector.tensor_mul(out=pt, in0=pt, in1=dm_sb[:, idx, :])
                for qc in range(2):
                    lhsT = pt[:, qc * P : (qc + 1) * P]
                    for dh in range(2):
                        nc.tensor.matmul(
                            h[qc][:, dh, :],
                            lhsT=lhsT,
                            rhs=v_t[:, dh * 512 : (dh + 1) * 512],
                            start=(j == 0),
                            stop=(j == jmax),
                        )
                    nc.tensor.matmul(
                        sums[:, qc : qc + 1],
                        lhsT=lhsT,
                        rhs=ones_sb,
                        start=False,
                        stop=(j == jmax),
                        skip_group_check=True,
                    )

            for s in range(NSLOTS):
                visits = [("diag", 0), ("diag", 1)] + [
                    ("gath", b) for b in range(16 * s + NBIN)
                ]
                jmax = len(visits) - 1
                h = [
                    hpsum.tile([P, 2, 512], F32, tag=f"hq{qc}", name=f"h{qc}_{s}")
                    for qc in range(2)
                ]
                sums = spsum.tile([P, 2], F32, tag="sums", name="sums")
                nc.tensor.matmul(
                    sums,
                    lhsT=zcol_sb,
                    rhs=zrow_sb,
                    start=True,
                    stop=False,
                    skip_group_check=True,
                )
                # software pipeline: logits of visit j+1 are emitted before pv of j
                kt_t, v_t = load_visit(s, *visits[0])
                lt_prev = logits(s, kt_t)
                v_prev = v_t
                for j in range(1, len(visits)):
                    kt_t, v_t = load_visit(s, *visits[j])
                    lt = logits(s, kt_t)
                    pv(s, j - 1, *visits[j - 1], lt_prev, v_prev, h, sums, jmax)
                    if s == 0 and j == 1:
                        q_proj_filler()
                    lt_prev, v_prev = lt, v_t
                pv(s, jmax, *visits[jmax], lt_prev, v_prev, h, sums, jmax)

                # ---- epilogue (chained per qc so qc0's output projection
                # overlaps qc1's vector/scalar work) -----------------------
                gt_sb = epool.tile([P, 8, 256], BF16, tag="gt", name="gt")
                for qc in range(2):
                    recip = epool.tile([P, 1], F32, tag="recip", name="recip")
                    nc.vector.reciprocal(out=recip, in_=sums[:, qc : qc + 1])
                    g32 = epool.tile([P, 2, 512], F32, tag="g32", name="g32")
                    nc.vector.tensor_scalar_mul(out=g32, in0=h[qc], scalar1=recip)
                    gb = gpool.tile([P, 1024], BF16, tag=f"g{qc}", name=f"g{qc}")
                    nc.scalar.activation(
                        out=gb, in_=g32.rearrange("p a b -> p (a b)"), func=AF.Silu
                    )
                    # transpose G -> gt [d-part, m, qc*P:...]
                    for m in range(8):
                        tp = tppsum.tile([P, 256], BF16, tag="tp", name="tp")
                        nc.tensor.transpose(
                            tp[:, :P],
                            gb[:, m * P : (m + 1) * P],
                            ident_sb,
                        )
                        nc.vector.tensor_copy(
                            out=gt_sb[:, m, qc * P : (qc + 1) * P], in_=tp[:, :P]
                        )
                    # output projection: O[q, d] via lhsT = gt chunks
                    op = hpsum.tile(
                        [P, 2, 512], F32, tag=f"hq{qc}", name=f"o{qc}_{s}"
                    )
                    for m in range(8):
                        for dh in range(2):
                            nc.tensor.matmul(
                                op[:, dh, :],
                                lhsT=gt_sb[:, m, qc * P : (qc + 1) * P],
                                rhs=wv2_sb[:, m, dh * 512 : (dh + 1) * 512],
                                start=(m == 0),
                                stop=(m == 7),
                            )
                    oo = epool.tile([P, 2, 512], F32, tag="oo", name="oo")
                    nc.vector.tensor_copy(out=oo, in_=op)
                    nc.sync.dma_start(
                        out=o_ext[s, qc], in_=oo.rearrange("p a b -> p (a b)")
                    )

        singles_ctx.__exit__(None, None, None)

    nc.finalize()
    return nc


_NC_CACHE = {}


def get_nc():
    if "nc" not in _NC_CACHE:
        _NC_CACHE["nc"] = build_kernel()
    return _NC_CACHE["nc"]


def build_dmask():
    p = np.arange(P)[:, None]
    u = np.arange(256)[None, :]
    m0 = (p <= u).astype(np.float32)
    m1 = (p + P <= u).astype(np.float32)
    return np.stack([m0, m1], axis=1).astype(ml_dtypes.bfloat16)  # [P, 2, 256]


def build_bias(c):
    """exp-bias per gathered-masked visit: slot s, i = block - 16s in [0, 14):
    visible on core c iff i < 2c."""
    row = np.empty(NSLOTS * NBIN, np.float32)
    for s in range(NSLOTS):
        for i in range(NBIN):
            row[s * NBIN + i] = 0.0 if i < 2 * c else -30000.0
    return np.broadcast_to(row, (P, NSLOTS * NBIN)).copy()


def build_in_maps(x, wq, wk, wv1, wv2):
    bf = ml_dtypes.bfloat16
    f8 = ml_dtypes.float8_e4m3fn
    xT = np.ascontiguousarray(np.asarray(x, np.float32).T)
    dmask = build_dmask()

    def to8(a):
        return np.clip(np.asarray(a, np.float32), -240, 240).astype(f8)

    w = {
        "wq8": to8(np.asarray(wq, np.float32) * KSCALE),
        "wk8": to8(np.asarray(wk, np.float32) * KSCALE),
        "wv1": np.asarray(wv1, np.float32).astype(bf),
        "wv2": np.asarray(wv2, np.float32).astype(bf),
        "dmask": dmask,
    }
    in_maps = []
    for c in range(NCORES):
        xq_c = np.concatenate(
            [xT[:, 256 * (c + 8 * s) : 256 * (c + 8 * s) + 256] for s in range(NSLOTS)],
            axis=1,
        )
        in_maps.append(
            {
                "xq8": to8(xq_c),
                "xqb": np.ascontiguousarray(xq_c).astype(bf),
                "bias": build_bias(c),
                **w,
            }
        )
    return in_maps


def assemble_out(results):
    out = np.empty((SEQ, D), np.float32)
    for c in range(NCORES):
        o = results[c]["o"]  # [4, 2, 128, 1024]
        for s in range(NSLOTS):
            r0 = 256 * (c + 8 * s)
            out[r0 : r0 + P, :] = o[s, 0]
            out[r0 + P : r0 + 256, :] = o[s, 1]
    return out


def kernel(x, wq, wk, wv1, wv2):
    in_maps = build_in_maps(x, wq, wk, wv1, wv2)
    nc = get_nc()
    res = run_bass_kernel_spmd(nc, in_maps, list(range(NCORES)))
    return assemble_out(res.results)
